# revision 10
# baseline (speedup 1.0000x reference)
"""Bidirectional LSTM (B=64, T=256, D=512, U=500) on 8 Trainium2 NeuronCores.

Sharding: 2 directions x 4 batch-groups -> 16 samples per core, one direction
per core. Backward cores receive time-reversed x from the host, so the device
program is pure SPMD (identical on all 8 cores).

Per-core program:
  Phase 1 (GEMM): xz[t*16+b, 4U] = x @ Wk + b     (f32r matmuls, K=512, M=4096, N=2000)
  Phase 2 (recurrence), 256 steps:
      z = xz[t] + h @ Wr        (f32r matmuls: lhsT = hT chunks [125,16], rhs = Wr)
      i,f,g,o = sigmoid/tanh gate slices (Keras order i,f,g,o)
      c = f*c + i*g ; h = o*tanh(c)
      hT via PE transpose for the next step's matmul
"""

import numpy as np

B, T, D, U = 64, 256, 512, 500
G4 = 4 * U            # 2000
NCORES = 8
BC = B // 4           # 16 samples per core
KCH, KQ = 4, 125      # U = 4 chunks of 125 (recurrent contraction)
DCH = 4               # D = 4 chunks of 128 (input contraction)
NSL = 500             # gate-slice / PSUM-bank width (<=512 fp32)
MT = (T * BC) // 128  # 32 M-tiles of 128 rows in the input GEMM

_CACHE = {}


def _build_program():
    import concourse.bass as bass
    import concourse.bacc as bacc
    import concourse.tile as tile
    import concourse.mybir as mybir
    from concourse.masks import make_identity

    dt = mybir.dt
    AF = mybir.ActivationFunctionType
    f32 = dt.float32
    f32r = dt.float32r

    nc = bacc.Bacc("TRN2")

    xT = nc.dram_tensor("xT", [D, T * BC], f32r, kind="ExternalInput")  # (d, t*16+b)
    h0 = nc.dram_tensor("h0", [BC, U], f32, kind="ExternalInput")
    c0 = nc.dram_tensor("c0", [BC, U], f32, kind="ExternalInput")
    Wk = nc.dram_tensor("Wk", [D, G4], f32r, kind="ExternalInput")
    Wr = nc.dram_tensor("Wr", [U, G4], f32r, kind="ExternalInput")
    bv = nc.dram_tensor("b", [G4], f32, kind="ExternalInput")
    y = nc.dram_tensor("y", [T, BC, U], f32, kind="ExternalOutput")
    xzo = nc.dram_tensor("xzbuf", [T * BC, G4], f32)

    with tile.TileContext(nc) as tc:
        with tc.tile_pool(name="dram", bufs=1, space="DRAM") as dpool, \
             tc.tile_pool(name="persist", bufs=1) as persist:
            xz = xzo

            # Wr chunks stay resident for the whole kernel: chunk k = Wr[125k:125k+125, :]
            wr_sb = persist.tile([KQ, KCH, G4], f32r)
            for k in range(KCH):
                nc.gpsimd.dma_start(wr_sb[:, k, :], Wr[k * KQ:(k + 1) * KQ, :])
            ident = persist.tile([BC, BC], f32)
            make_identity(nc, ident)

            # ---------------- Phase 1: xz = x @ Wk + b ----------------
            with tc.tile_pool(name="gx", bufs=1) as gx, \
                 tc.tile_pool(name="gpsum", bufs=2, space="PSUM") as gps, \
                 tc.tile_pool(name="gout", bufs=3) as gout:
                xT_sb = gx.tile([128, DCH, T * BC], f32r)
                wk_sb = gx.tile([128, DCH, G4], f32r)
                for k in range(DCH):
                    nc.gpsimd.dma_start(xT_sb[:, k, :], xT[k * 128:(k + 1) * 128, :])
                    nc.gpsimd.dma_start(wk_sb[:, k, :], Wk[k * 128:(k + 1) * 128, :])
                b_bc = gx.tile([128, G4], f32)
                bva = bv[:]
                nc.gpsimd.dma_start(
                    b_bc, bass.AP(bva.tensor, bva.offset, [[0, 128], [1, G4]])
                )
                for m in range(MT):
                    ps = gps.tile([128, 4, 512], f32)
                    for n in range(4):
                        for k in range(DCH):
                            nc.tensor.matmul(
                                ps[:, n, 0:NSL],
                                lhsT=xT_sb[:, k, m * 128:(m + 1) * 128],
                                rhs=wk_sb[:, k, n * NSL:(n + 1) * NSL],
                                start=(k == 0),
                                stop=(k == DCH - 1),
                            )
                    so = gout.tile([128, G4], f32)
                    for n in range(4):
                        nc.vector.tensor_add(
                            so[:, n * NSL:(n + 1) * NSL],
                            ps[:, n, 0:NSL],
                            b_bc[:, n * NSL:(n + 1) * NSL],
                        )
                    nc.sync.dma_start(xz[m * 128:(m + 1) * 128, :], so)

            # ---------------- Phase 2: recurrence ----------------
            with tc.tile_pool(name="state", bufs=2) as st, \
                 tc.tile_pool(name="gates", bufs=2) as gt, \
                 tc.tile_pool(name="xzin", bufs=4) as xzp, \
                 tc.tile_pool(name="rpsum", bufs=1, space="PSUM") as rps, \
                 tc.tile_pool(name="tpsum", bufs=4, space="PSUM") as tps:

                h_sb = st.tile([BC, U], f32, tag="h")
                c_sb = st.tile([BC, U], f32, tag="c")
                nc.sync.dma_start(h_sb, h0[:, :])
                nc.sync.dma_start(c_sb, c0[:, :])
                hT_sb = st.tile([KQ, KCH, BC], f32r, tag="hT")
                for j in range(KCH):
                    tp = tps.tile([KQ, BC], f32)
                    nc.tensor.transpose(tp, h_sb[:, j * KQ:(j + 1) * KQ], ident)
                    nc.vector.tensor_copy(hT_sb[:, j, :], tp)

                for t in range(T):
                    xzt = xzp.tile([BC, G4], f32)
                    nc.sync.dma_start(xzt, xz[t * BC:(t + 1) * BC, :])
                    ps = rps.tile([BC, 4, 512], f32)
                    for n in range(4):
                        for k in range(KCH):
                            nc.tensor.matmul(
                                ps[:, n, 0:NSL],
                                lhsT=hT_sb[:, k, :],
                                rhs=wr_sb[:, k, n * NSL:(n + 1) * NSL],
                                start=(k == 0),
                                stop=(k == KCH - 1),
                            )
                    s = gt.tile([BC, G4], f32, tag="s")
                    a = gt.tile([BC, G4], f32, tag="a")
                    for n in range(4):
                        nc.vector.tensor_add(
                            s[:, n * NSL:(n + 1) * NSL],
                            ps[:, n, 0:NSL],
                            xzt[:, n * NSL:(n + 1) * NSL],
                        )
                    # Keras gate order i, f, g, o: sigmoid(i,f), tanh(g), sigmoid(o)
                    nc.scalar.activation(a[:, 0:1000], s[:, 0:1000], AF.Sigmoid)
                    nc.scalar.activation(a[:, 1000:1500], s[:, 1000:1500], AF.Tanh)
                    nc.scalar.activation(a[:, 1500:2000], s[:, 1500:2000], AF.Sigmoid)

                    t1 = st.tile([BC, U], f32, tag="t1")
                    t2 = st.tile([BC, U], f32, tag="t2")
                    nc.vector.tensor_mul(t1, a[:, 0:500], a[:, 1000:1500])
                    nc.vector.tensor_mul(t2, a[:, 500:1000], c_sb)
                    c_new = st.tile([BC, U], f32, tag="c")
                    nc.vector.tensor_add(c_new, t1, t2)
                    th = st.tile([BC, U], f32, tag="th")
                    nc.scalar.activation(th, c_new, AF.Tanh)
                    h_new = st.tile([BC, U], f32, tag="h")
                    nc.vector.tensor_mul(h_new, a[:, 1500:2000], th)
                    nc.sync.dma_start(y[t], h_new)

                    hT_new = st.tile([KQ, KCH, BC], f32r, tag="hT")
                    for j in range(KCH):
                        tp = tps.tile([KQ, BC], f32)
                        nc.tensor.transpose(tp, h_new[:, j * KQ:(j + 1) * KQ], ident)
                        nc.vector.tensor_copy(hT_new[:, j, :], tp)
                    h_sb, c_sb, hT_sb = h_new, c_new, hT_new
    nc.finalize()
    return nc


def _make_in_maps(x, h_f, c_f, h_b, c_b, Wk_f, Wr_f, b_f, Wk_b, Wr_b, b_b):
    x = np.ascontiguousarray(np.asarray(x, np.float32))
    in_maps = []
    for core in range(NCORES):
        d = core // 4           # 0 = forward, 1 = backward
        g = core % 4
        bs = slice(g * BC, (g + 1) * BC)
        xc = x[bs] if d == 0 else x[bs, ::-1]
        # xT[d, t*16+b] = xc[b, t, d]
        xTc = np.ascontiguousarray(xc.transpose(2, 1, 0).reshape(D, T * BC))
        in_maps.append({
            "xT": xTc,
            "h0": np.ascontiguousarray((h_f if d == 0 else h_b)[bs], np.float32),
            "c0": np.ascontiguousarray((c_f if d == 0 else c_b)[bs], np.float32),
            "Wk": np.ascontiguousarray(Wk_f if d == 0 else Wk_b, np.float32),
            "Wr": np.ascontiguousarray(Wr_f if d == 0 else Wr_b, np.float32),
            "b": np.ascontiguousarray(b_f if d == 0 else b_b, np.float32),
        })
    return in_maps


def kernel(x, h_f, c_f, h_b, c_b, Wk_f, Wr_f, b_f, Wk_b, Wr_b, b_b):
    from concourse.bass_utils import run_bass_kernel_spmd

    if "nc" not in _CACHE:
        _CACHE["nc"] = _build_program()
    nc = _CACHE["nc"]
    in_maps = _make_in_maps(x, h_f, c_f, h_b, c_b, Wk_f, Wr_f, b_f, Wk_b, Wr_b, b_b)

    import os
    trace = os.environ.get("BLSTM_TRACE") == "1"
    tmpdir = os.environ.get("BLSTM_TRACE_DIR") or None
    br = run_bass_kernel_spmd(nc, in_maps, list(range(NCORES)), trace=trace, tmpdir=tmpdir)
    _CACHE["exec_time_ns"] = br.exec_time_ns
    res = br.results

    out = np.empty((B, T, 2 * U), np.float32)
    for core in range(NCORES):
        d = core // 4
        g = core % 4
        yc = res[core]["y"]                    # [T, BC, U]
        yc = np.transpose(yc, (1, 0, 2))       # [BC, T, U]
        bs = slice(g * BC, (g + 1) * BC)
        if d == 0:
            out[bs, :, :U] = yc
        else:
            out[bs, :, U:] = yc[:, ::-1]
    return out
